# revision 15
# baseline (speedup 1.0000x reference)
"""Multi-head attention (B=4, S=2048, D=1024, H=16) on 8 trn2 NeuronCores.

Sharding: core c handles batch b = c//2, sequence half = c%2 (1024 query rows).
K/V are computed for the full sequence on each core (duplicated across the
half-pair) so no cross-core communication is needed; output rows are disjoint
and the host gather is a pure concatenation.

Layout strategy ("transposed scores"): all projections that feed the score
matmul are computed transposed (QT/KT = [d_model, seq] with d on partitions),
so scores come out as scoresT = [k_seq on partitions, q_seq on free].  The
softmax sum over k is obtained for free by appending a ones-column to V
(M=65 context matmul); exp() is ScalarE reading PSUM directly.  No on-chip
transposes anywhere.  All matmuls in bf16 with fp32 PSUM accumulation.
"""

import numpy as np
import ml_dtypes

B, S, D = 4, 2048, 1024
H, DH = 16, 64
NCORES = 8
SH = S // 2          # query rows per core
P = 128
KT_IN = D // P       # 8 contraction tiles for the projections
NPAIR = H // 2       # 8 head pairs
NKT = S // P         # 16 k-tiles in the attention contraction
QCH = SH // 512      # 2 q-chunks per core
SCH = S // 512       # 4 s-chunks for the K/V projections
VW = DH + 1          # 65: V columns per head incl. the ones column

BF16 = ml_dtypes.bfloat16

_NC_CACHE = {}


def _build_nc(phases="all"):
    import concourse.bass as bass
    import concourse.mybir as mybir
    import concourse.tile as tile
    from concourse import bacc
    from contextlib import ExitStack

    dt = mybir.dt
    F32, BF = dt.float32, dt.bfloat16
    AF = mybir.ActivationFunctionType
    ALU = mybir.AluOpType

    nc = bacc.Bacc(None)

    qT_d = nc.dram_tensor("qT", [KT_IN, P, SH], BF, kind="ExternalInput")
    kT_d = nc.dram_tensor("kT", [KT_IN, P, S], BF, kind="ExternalInput")
    vT_d = nc.dram_tensor("vT", [KT_IN, P, S], BF, kind="ExternalInput")
    wq_d = nc.dram_tensor("wq", [KT_IN, P, D], BF, kind="ExternalInput")
    wk_d = nc.dram_tensor("wk", [KT_IN, P, D], BF, kind="ExternalInput")
    wv_d = nc.dram_tensor("wv", [KT_IN, P, D], BF, kind="ExternalInput")
    wo_d = nc.dram_tensor("wo", [P, NPAIR, D], BF, kind="ExternalInput")
    bq_d = nc.dram_tensor("bq", [P, KT_IN], F32, kind="ExternalInput")
    bk_d = nc.dram_tensor("bk", [P, KT_IN], F32, kind="ExternalInput")
    bo_d = nc.dram_tensor("bo", [P, D], BF, kind="ExternalInput")
    out_d = nc.dram_tensor("out", [SH, D], F32, kind="ExternalOutput")

    with tile.TileContext(nc) as tc, ExitStack() as ctx:
        persist = ctx.enter_context(tc.tile_pool(name="persist", bufs=1))
        psum_pr = ctx.enter_context(
            tc.tile_pool(name="psum_pr", bufs=2, space="PSUM")
        )
        psum_sc = ctx.enter_context(
            tc.tile_pool(name="psum_sc", bufs=2, space="PSUM")
        )
        psum_cx = ctx.enter_context(
            tc.tile_pool(name="psum_cx", bufs=2, space="PSUM")
        )

        # ---- persistent SBUF tensors
        QT = persist.tile([P, KT_IN, SH], BF)      # [128(d of pair), pair, q]
        KT = persist.tile([P, KT_IN, S], BF)       # [128(d of pair), pair, k]
        VA = persist.tile([P, NKT, H * VW], BF)    # [128(k), s-tile, 65*h + d]
        CT = persist.tile([P, NPAIR, SH], BF)      # ctxT, pair-packed rows
        BQ = persist.tile([P, KT_IN], F32)
        BK = persist.tile([P, KT_IN], F32)

        nc.sync.dma_start(BQ, bq_d[:])
        nc.sync.dma_start(BK, bk_d[:])
        # ones columns of V_aug (col 64 of each head's 65-wide block)
        va_h = VA[:].rearrange("p t (h e) -> p t h e", e=VW)
        nc.vector.memset(va_h[:, :, :, DH : DH + 1], 1.0)

        with (
            tc.tile_pool(name="wproj", bufs=1) as wproj,
            tc.tile_pool(name="stream", bufs=2) as stream,
            tc.tile_pool(name="vstream", bufs=3) as vstream,
        ):
            WQ = wproj.tile([P, KT_IN, D], BF)
            WK = wproj.tile([P, KT_IN, D], BF)
            WV = wproj.tile([P, KT_IN, D], BF)
            nc.sync.dma_start(WQ, wq_d[:].rearrange("k p d -> p k d"))
            nc.sync.dma_start(WK, wk_d[:].rearrange("k p d -> p k d"))
            nc.sync.dma_start(WV, wv_d[:].rearrange("k p d -> p k d"))

            # ---- QT projection: QT[:, mt, c*512:+512] = (Wq.T @ q.T) + bq
            for c in range(QCH):
                qs = stream.tile([P, KT_IN, 512], BF, tag="qs")
                nc.sync.dma_start(
                    qs,
                    qT_d[:, :, c * 512 : (c + 1) * 512].rearrange("k p s -> p k s"),
                )
                for mt in range(KT_IN):
                    ps = psum_pr.tile([P, 512], F32, tag="prps")
                    for kt in range(KT_IN):
                        nc.tensor.matmul(
                            ps,
                            lhsT=WQ[:, kt, mt * P : (mt + 1) * P],
                            rhs=qs[:, kt, :],
                            start=(kt == 0),
                            stop=(kt == KT_IN - 1),
                        )
                    nc.scalar.activation(
                        QT[:, mt, c * 512 : (c + 1) * 512],
                        ps,
                        AF.Identity,
                        bias=BQ[:, mt : mt + 1],
                        scale=1.0,
                    )

            # ---- KT projection (full sequence)
            for c in range(SCH):
                ks = stream.tile([P, KT_IN, 512], BF, tag="ks")
                nc.sync.dma_start(
                    ks,
                    kT_d[:, :, c * 512 : (c + 1) * 512].rearrange("k p s -> p k s"),
                )
                for mt in range(KT_IN):
                    ps = psum_pr.tile([P, 512], F32, tag="prps")
                    for kt in range(KT_IN):
                        nc.tensor.matmul(
                            ps,
                            lhsT=WK[:, kt, mt * P : (mt + 1) * P],
                            rhs=ks[:, kt, :],
                            start=(kt == 0),
                            stop=(kt == KT_IN - 1),
                        )
                    nc.scalar.activation(
                        KT[:, mt, c * 512 : (c + 1) * 512],
                        ps,
                        AF.Identity,
                        bias=BK[:, mt : mt + 1],
                        scale=1.0,
                    )

            # ---- V projection into V_aug ([s,d] layout, 65-strided head blocks)
            for st in range(NKT):
                vs = vstream.tile([P, KT_IN, P], BF, tag="vs")
                nc.sync.dma_start(
                    vs, vT_d[:, :, st * P : (st + 1) * P].rearrange("k p s -> p k s")
                )
                for dc in range(2):
                    ps = psum_pr.tile([P, 512], F32, tag="prps")
                    for kt in range(KT_IN):
                        nc.tensor.matmul(
                            ps,
                            lhsT=vs[:, kt, :],
                            rhs=WV[:, kt, dc * 512 : (dc + 1) * 512],
                            start=(kt == 0),
                            stop=(kt == KT_IN - 1),
                        )
                    # heads 8*dc .. 8*dc+7 live at cols 65*h .. 65*h+63
                    dst = va_h[:, st, 8 * dc : 8 * dc + 8, 0:DH]
                    nc.vector.tensor_copy(
                        dst, ps[:].rearrange("p (h e) -> p h e", e=DH)
                    )

        # ---- attention, one head-pair at a time
        inv_sqrt_dh = 1.0 / float(np.sqrt(DH))
        attn_ctx = ExitStack()
        ppool = attn_ctx.enter_context(tc.tile_pool(name="ppool", bufs=4))
        small = attn_ctx.enter_context(tc.tile_pool(name="small", bufs=2))
        pair_range = range(NPAIR) if phases in ("all", "attn") else range(0)
        for j in pair_range:
            for c in range(QCH):
                cx_ps = []
                for v in range(2):
                    cxt = psum_cx.tile([P, 512], F32, tag="cxps")
                    cx_ps.append(cxt)
                for g in range(NKT // 2):  # groups of 2 k-tiles
                    for v in range(2):  # head within pair
                        lo = 64 * v
                        sc = psum_sc.tile([P, 2, 512], F32, tag="scps")
                        for t in range(2):
                            kt = 2 * g + t
                            nc.tensor.matmul(
                                sc[:, t, :],
                                lhsT=KT[lo : lo + 64, j, kt * P : (kt + 1) * P],
                                rhs=QT[lo : lo + 64, j, c * 512 : (c + 1) * 512],
                                start=True,
                                stop=True,
                            )
                        pt = ppool.tile([P, 2, 512], BF, tag="pt")
                        nc.scalar.activation(pt, sc, AF.Exp, scale=inv_sqrt_dh)
                        h = 2 * j + v
                        for t in range(2):
                            kt = 2 * g + t
                            nc.tensor.matmul(
                                cx_ps[v][0 : DH + 1, :],
                                lhsT=VA[:, kt, VW * h : VW * h + VW],
                                rhs=pt[:, t, :],
                                start=(kt == 0),
                                stop=(kt == NKT - 1),
                            )
                # normalize: ctxT = ctx_unnorm * (1/rowsum), write into CT
                for v in range(2):
                    rec = small.tile([1, 512], F32, tag="rec")
                    nc.vector.reciprocal(rec, cx_ps[v][DH : DH + 1, :])
                    recb = small.tile([DH, 512], F32, tag="recb")
                    nc.gpsimd.partition_broadcast(recb, rec)
                    if v == 0:
                        nc.vector.tensor_tensor(
                            CT[0:DH, j, c * 512 : (c + 1) * 512],
                            cx_ps[v][0:DH, :],
                            recb,
                            ALU.mult,
                        )
                    else:
                        stg = small.tile([DH, 512], BF, tag="stg")
                        nc.vector.tensor_tensor(
                            stg, cx_ps[v][0:DH, :], recb, ALU.mult
                        )
                        nc.sync.dma_start(
                            CT[DH:P, j, c * 512 : (c + 1) * 512], stg
                        )

        attn_ctx.close()

        if phases != "all":
            # debug variants: dump whatever QT holds so the output is written
            dbg = ctx.enter_context(tc.tile_pool(name="dbg", bufs=2))
            for mt in range(KT_IN):
                dt_ = dbg.tile([P, SH], F32, tag="dbg")
                nc.vector.tensor_copy(dt_, QT[:, mt, :])
                nc.sync.dma_start(out_d[mt * P : (mt + 1) * P, :], dt_)
        else:
            # ---- output projection: out[qt, ec] = sum_h ctxT_h.T @ Wo_h + bo
            wout = ctx.enter_context(tc.tile_pool(name="wout", bufs=1))
            ostream = ctx.enter_context(tc.tile_pool(name="ostream", bufs=3))
            WO = wout.tile([P, NPAIR, D], BF)
            BO = wout.tile([P, D], BF)
            ONES = wout.tile([P, P], BF)
            nc.sync.dma_start(WO, wo_d[:])
            nc.sync.dma_start(BO, bo_d[:])
            nc.vector.memset(ONES, 1.0)
            for qt in range(SH // P):
                for ec in range(2):
                    ps = psum_pr.tile([P, 512], F32, tag="prps")
                    nc.tensor.matmul(
                        ps,
                        lhsT=ONES,
                        rhs=BO[:, ec * 512 : (ec + 1) * 512],
                        start=True,
                        stop=False,
                    )
                    for j in range(NPAIR):
                        nc.tensor.matmul(
                            ps,
                            lhsT=CT[:, j, qt * P : (qt + 1) * P],
                            rhs=WO[:, j, ec * 512 : (ec + 1) * 512],
                            start=False,
                            stop=(j == NPAIR - 1),
                        )
                    ot = ostream.tile([P, 512], F32, tag="ot")
                    nc.vector.tensor_copy(ot, ps)
                    nc.sync.dma_start(
                        out_d[qt * P : (qt + 1) * P, ec * 512 : (ec + 1) * 512], ot
                    )

    nc.compile()
    return nc


def _get_nc():
    if "nc" not in _NC_CACHE:
        import os

        _NC_CACHE["nc"] = _build_nc(os.environ.get("KERNEL_PHASES", "all"))
    return _NC_CACHE["nc"]


def kernel(query, key, value, Wq, bq, Wk, bk, Wv, bv, Wo, bo):
    from concourse.bass_utils import run_bass_kernel_spmd

    query = np.asarray(query, dtype=np.float32)
    key = np.asarray(key, dtype=np.float32)
    value = np.asarray(value, dtype=np.float32)
    Wq = np.asarray(Wq, dtype=np.float32)
    Wk = np.asarray(Wk, dtype=np.float32)
    Wv = np.asarray(Wv, dtype=np.float32)
    Wo = np.asarray(Wo, dtype=np.float32)
    bq = np.asarray(bq, dtype=np.float32)
    bk = np.asarray(bk, dtype=np.float32)
    bv = np.asarray(bv, dtype=np.float32)
    bo = np.asarray(bo, dtype=np.float32)

    nc = _get_nc()

    wq_t = np.ascontiguousarray(Wq.reshape(KT_IN, P, D)).astype(BF16)
    wk_t = np.ascontiguousarray(Wk.reshape(KT_IN, P, D)).astype(BF16)
    wv_t = np.ascontiguousarray(Wv.reshape(KT_IN, P, D)).astype(BF16)
    # Wo row (h*64+d) -> [ (v,d)=128, pair j, e ]
    wo_p = np.ascontiguousarray(
        Wo.reshape(NPAIR, 2, DH, D).transpose(1, 2, 0, 3).reshape(P, NPAIR, D)
    ).astype(BF16)
    bq_t = np.ascontiguousarray(bq.reshape(KT_IN, P).T).astype(np.float32)
    bk_t = np.ascontiguousarray(bk.reshape(KT_IN, P).T).astype(np.float32)
    # attn rows sum to 1 => the V bias contributes bv @ Wo, constant per output
    bo_eff = (bo.astype(np.float64) + bv.astype(np.float64) @ Wo.astype(np.float64))
    bo_rep = np.tile((bo_eff / P)[None, :], (P, 1)).astype(BF16)

    shared = {
        "wq": wq_t, "wk": wk_t, "wv": wv_t, "wo": wo_p,
        "bq": bq_t, "bk": bk_t, "bo": bo_rep,
    }
    in_maps = []
    for c in range(NCORES):
        b, hf = divmod(c, 2)
        qT = np.ascontiguousarray(
            query[b, hf * SH : (hf + 1) * SH, :].T
        ).reshape(KT_IN, P, SH).astype(BF16)
        kT = np.ascontiguousarray(key[b].T).reshape(KT_IN, P, S).astype(BF16)
        vT = np.ascontiguousarray(value[b].T).reshape(KT_IN, P, S).astype(BF16)
        in_maps.append({**shared, "qT": qT, "kT": kT, "vT": vT})

    _NC_CACHE["last_in_maps"] = in_maps
    globals()["_LAST_IN_MAPS"] = in_maps
    res = run_bass_kernel_spmd(nc, in_maps, core_ids=list(range(NCORES)))

    out = np.empty((B, S, D), np.float32)
    for c in range(NCORES):
        b, hf = divmod(c, 2)
        out[b, hf * SH : (hf + 1) * SH, :] = res.results[c]["out"]
    return out
